# revision 1
# baseline (speedup 1.0000x reference)
"""Trainium2 Bass kernel for nn_PlainDecoder (2-layer bidirectional-style GRU
decoder + vocab projection + log_softmax).

Sharding:
  - GRU scan: data-parallel over batch (32 batches -> 4 per core). Each core
    runs both "directions" of both layers for its 4 batches. No collectives.
  - Logits/log_softmax: vocab-parallel. x2 (GRU output) is AllGather'd so every
    core sees all 4096 (b,s) positions; each core computes logits for its
    4096-wide vocab shard (32768 padded vocab / 8). Row-wise sum(exp(l)) is
    AllGather'd per position-block and reduced on-chip, then out = l - ln(S)
    is written directly. No logits round-trip through DRAM.

Scan layout: gates are "packed" -- the host permutes each direction's 1536
gate columns into 4 groups of 384 = [r-sub(128) | z-sub(128) | n-sub(128)],
and the gate matmuls use PE column-tiling so group j lands on PSUM partitions
32j+b. The GRU cell elementwise chain then runs on [128, 128..256]-shaped
tiles (128 partitions busy) instead of [4, 512..1024] (4 partitions busy).
Gi and bias additions ride on the PE as K=4/K=1 accumulating matmuls.

Matmul operands are float32r (full PE rate, fp32 storage).
"""

import os
import sys
from contextlib import ExitStack

for _p in ("/opt/trn_rl_repo", "/root/.axon_site/_ro/trn_rl_repo"):
    if os.path.isdir(_p) and _p not in sys.path:
        sys.path.insert(0, _p)

import numpy as np  # noqa: E402

V, E, H, L, B, S = 32000, 512, 512, 2, 32, 128
NC_ = 8                      # cores
BPC = B // NC_               # batches per core = 4
R = BPC * S                  # rows per core = 512 (s-major: row = 4*t + b)
G = 3 * H                    # 1536 gates per dir
GG = 384                     # packed gate-group width (128 r | 128 z | 128 n)
VPAD = 32768
VS = VPAD // NC_             # vocab shard per core = 4096
NEG = -80.0                  # pad bias -> exp() ~ 1.8e-35, ln finite

_BUILT = {}

# packed gate permutation: new col j*384 + p*128 + i <- old col p*512 + j*128 + i
_PERM = np.concatenate(
    [np.concatenate([np.arange(p * 512 + j * 128, p * 512 + j * 128 + 128)
                     for p in range(3)]) for j in range(4)])


def _build_nc(T=S, n_cores=NC_, sim=False, nblk_lim=None, skip_gi=False):
    """Build the Bass program (same NEFF for all cores; per-core data differs).

    sim=True replaces collectives with local DMAs so TimelineSim can run.
    """
    import concourse.bass as bass  # noqa: F401
    import concourse.mybir as mybir
    import concourse.tile as tile
    from concourse import bacc
    from concourse.masks import make_identity

    dt = mybir.dt
    f32 = dt.float32
    fr = dt.float32r
    AF = mybir.ActivationFunctionType
    OP = mybir.AluOpType

    nc = bacc.Bacc("TRN2", target_bir_lowering=False, debug=False,
                   num_devices=n_cores)

    # ---------------- DRAM I/O ----------------
    embT = nc.dram_tensor("embT", [128, 4, R], fr, kind="ExternalInput")
    h0T = nc.dram_tensor("h0T", [128, 4, 2, 2, BPC], fr, kind="ExternalInput")
    WihT0 = nc.dram_tensor("WihT0", [128, 4, 2, G], fr, kind="ExternalInput")
    WhhT0 = nc.dram_tensor("WhhT0", [128, 4, 2, G], fr, kind="ExternalInput")
    bGi0 = nc.dram_tensor("bGi0", [1, 2, G], fr, kind="ExternalInput")
    bHh0 = nc.dram_tensor("bHh0", [1, 2, G], fr, kind="ExternalInput")
    WihT1 = nc.dram_tensor("WihT1", [128, 8, 2, G], fr, kind="ExternalInput")
    WhhT1 = nc.dram_tensor("WhhT1", [128, 4, 2, G], fr, kind="ExternalInput")
    bGi1 = nc.dram_tensor("bGi1", [1, 2, G], fr, kind="ExternalInput")
    bHh1 = nc.dram_tensor("bHh1", [1, 2, G], fr, kind="ExternalInput")
    fcwT = nc.dram_tensor("fcwT", [128, 8, VS], fr, kind="ExternalInput")
    fcb = nc.dram_tensor("fcb", [1, VS], fr, kind="ExternalInput")
    onesD = nc.dram_tensor("ones", [1, 512], fr, kind="ExternalInput")
    id4D = nc.dram_tensor("id4", [4, 4], fr, kind="ExternalInput")

    NROW = n_cores * R  # 4096 global rows
    out_d = nc.dram_tensor("out", [NROW, VS], f32, kind="ExternalOutput")

    # internal DRAM
    giD = nc.dram_tensor("giD", [4, 128, 2, G], fr, kind="Internal")
    agx_in = nc.dram_tensor("agx_in", [128, 8, R], fr, kind="Internal")
    agx_out = nc.dram_tensor("agx_out", [n_cores * 128, 8, R], fr,
                             kind="Internal", addr_space="Shared")
    NBLK = NROW // 128       # 32 position blocks
    ags_in = [nc.dram_tensor(f"ags_in{g}", [1, 128], f32, kind="Internal")
              for g in range(NBLK)]
    ags_out = [nc.dram_tensor(f"ags_out{g}", [n_cores, 128], f32,
                              kind="Internal", addr_space="Shared")
               for g in range(NBLK)]
    rg = [list(range(n_cores))]

    with tile.TileContext(nc) as tc, ExitStack() as top:
        constp = top.enter_context(tc.tile_pool(name="const", bufs=1))
        ones = constp.tile([1, 512], fr)
        nc.sync.dma_start(ones[:], onesD[:])
        ones8 = constp.tile([8, 1], f32)
        nc.vector.memset(ones8[:], 1.0)
        id4r = constp.tile([4, 4], fr)
        nc.sync.dma_start(id4r[:], id4D[:])
        ident4 = constp.tile([4, 4], f32)
        make_identity(nc, ident4[:])
        ident128 = constp.tile([128, 128], f32)
        make_identity(nc, ident128[:])

        with ExitStack() as scan_stack:
            wres = scan_stack.enter_context(tc.tile_pool(name="wres", bufs=1))
            histp = scan_stack.enter_context(tc.tile_pool(name="hist", bufs=1))

            # resident scan tensors (whh0/whh1 share one slot via same tag)
            whh0 = wres.tile([128, 4, 2, G], fr, tag="whh")
            nc.sync.dma_start(whh0[:], WhhT0[:])
            h0T_sb = wres.tile([128, 4, 2, 2, BPC], fr, tag="h0T")
            nc.sync.dma_start(h0T_sb[:], h0T[:])

            x1T = histp.tile([128, 8, R], fr, tag="x1T")    # layer0 out hist

            # ---------- Gi = x @ Wih.T + bGi -> giD ----------
            def gi_phase(xT_sb, WihD, bGiD, kc, suffix):
                with (
                    tc.tile_pool(name=f"giw{suffix}", bufs=1) as giw,
                    tc.tile_pool(name=f"gis{suffix}", bufs=2) as gisp,
                    tc.tile_pool(name=f"gip{suffix}", bufs=1,
                                 space="PSUM") as gips,
                ):
                    bgi = giw.tile([1, 2, G], fr, tag="bgi")
                    nc.sync.dma_start(bgi[:], bGiD[:])
                    wih = giw.tile([128, kc, 2, G], fr, tag="wih")
                    nc.sync.dma_start(wih[:], WihD[:])
                    for m in range(4):          # row chunks of 128
                        for d in range(2):
                            ps = gips.tile([128, 3, 512], f32, tag="gps",
                                           bufs=2)
                            for ni in range(3):
                                for k in range(kc):
                                    nc.tensor.matmul(
                                        ps[:, ni, :],
                                        xT_sb[:, k, 128 * m:128 * (m + 1)],
                                        wih[:, k, d, 512 * ni:512 * (ni + 1)],
                                        start=(k == 0), stop=False)
                                nc.tensor.matmul(
                                    ps[:, ni, :], ones[0:1, 0:128],
                                    bgi[0:1, d, 512 * ni:512 * (ni + 1)],
                                    start=False, stop=True)
                            stage = gisp.tile([128, G], fr, tag="stage")
                            nc.vector.tensor_copy(
                                stage[:],
                                ps[:].rearrange("p n f -> p (n f)"))
                            nc.sync.dma_start(giD[m, :, d, :], stage[:])

            with tc.tile_pool(name="gix", bufs=1) as gixp:
                embT_sb = gixp.tile([128, 4, R], fr, tag="embT")
                nc.sync.dma_start(embT_sb[:], embT[:])
                if not skip_gi:
                    gi_phase(embT_sb, WihT0, bGi0, 4, "0")

            # ---------- the two GRU scans (packed-gate layout) ----------
            def scan_layer(layer, whh, bHhD, histT):
                with (
                    tc.tile_pool(name=f"ps{layer}", bufs=1, space="PSUM") as psp,
                    tc.tile_pool(name=f"pt{layer}", bufs=2, space="PSUM") as pstp,
                    tc.tile_pool(name=f"ch{layer}", bufs=2) as chp,
                    tc.tile_pool(name=f"gs{layer}", bufs=3) as gslp,
                ):
                    bhh = chp.tile([1, 2, G], fr, tag="bhh", bufs=1)
                    nc.sync.dma_start(bhh[:], bHhD[:])
                    for t in range(T):
                        gsl = gslp.tile([BPC, 2, G], fr, tag="gsl")
                        nc.sync.dma_start(
                            gsl[:], giD[t // 32, 4 * t % 128:4 * t % 128 + BPC])
                        P = psp.tile([BPC, 2, 3, 512], f32, tag="P")
                        for d in range(2):
                            for ni in range(3):
                                for k in range(4):
                                    if t == 0:
                                        lhsT = h0T_sb[:, k, layer, d, :]
                                    else:
                                        lhsT = histT[:, 4 * d + k,
                                                     4 * (t - 1):4 * (t - 1) + 4]
                                    nc.tensor.matmul(
                                        P[:, d, ni, :], lhsT,
                                        whh[:, k, d, 512 * ni:512 * (ni + 1)],
                                        start=(k == 0), stop=False)
                                if ni == 2:
                                    # only the n-gate has a live b_hh part
                                    nc.tensor.matmul(
                                        P[:, d, ni, :], ones[0:1, 0:BPC],
                                        bhh[0:1, d, 512 * ni:512 * (ni + 1)],
                                        start=False, stop=True)
                                else:
                                    # gi for r,z accumulates on the PE
                                    nc.tensor.matmul(
                                        P[:, d, ni, :], id4r[:],
                                        gsl[:, d, 512 * ni:512 * (ni + 1)],
                                        start=False, stop=True,
                                        skip_group_check=True)
                        rzs = chp.tile([BPC, 2, 2 * H], f32, tag="rzs")
                        n1 = chp.tile([BPC, 2, H], f32, tag="n1")
                        nt = chp.tile([BPC, 2, H], f32, tag="nt")
                        d1 = chp.tile([128, 2, 4, BPC], f32, tag="d1")
                        for d in range(2):
                            nc.scalar.activation(
                                rzs[:, d, :],
                                P[:, d, 0:2, :].rearrange("p n f -> p (n f)"),
                                AF.Sigmoid)
                            # n = tanh(gi_n + r * gh_n)
                            nc.vector.tensor_mul(n1[:, d, :], P[:, d, 2, :],
                                                 rzs[:, d, 0:H])
                            nc.vector.tensor_add(n1[:, d, :], n1[:, d, :],
                                                 gsl[:, d, 1024:1536])
                            nc.scalar.activation(nt[:, d, :], n1[:, d, :],
                                                 AF.Tanh)
                            # transpose n and z into history (lhsT) layout
                            pt = pstp.tile([128, 2, 4, BPC], f32, tag="pt")
                            for k in range(4):
                                nc.tensor.transpose(
                                    pt[:, 0, k, :],
                                    nt[:, d, 128 * k:128 * (k + 1)],
                                    ident4[:])
                                nc.tensor.transpose(
                                    pt[:, 1, k, :],
                                    rzs[:, d, H + 128 * k:H + 128 * (k + 1)],
                                    ident4[:])
                            # h'T = nT + zT * (hT_prev - nT), straight into hist
                            hprev = (h0T_sb[:, :, layer, d, :] if t == 0
                                     else histT[:, 4 * d:4 * d + 4,
                                                4 * (t - 1):4 * (t - 1) + 4])
                            nc.vector.tensor_sub(d1[:, d], hprev, pt[:, 0])
                            nc.vector.tensor_mul(d1[:, d], d1[:, d], pt[:, 1])
                            nc.vector.tensor_add(
                                histT[:, 4 * d:4 * d + 4, 4 * t:4 * t + 4],
                                d1[:, d], pt[:, 0])

            scan_layer(0, whh0, bHh0, x1T)

            # ---------- Gi1 = x1 @ Wih1.T + bGi1 -> giD ----------
            if not skip_gi:
                gi_phase(x1T, WihT1, bGi1, 8, "1")

            # load layer-1 recurrent weights into the shared slot
            whh1 = wres.tile([128, 4, 2, G], fr, tag="whh")
            nc.sync.dma_start(whh1[:], WhhT1[:])

            x2T = histp.tile([128, 8, R], fr, tag="x2T")    # layer1 out hist
            scan_layer(1, whh1, bHh1, x2T)

            # ship x2 out for the AllGather before scan pools close
            nc.sync.dma_start(agx_in[:], x2T[:])

        if sim:
            nc.sync.dma_start(agx_out[0:128], agx_in[:])
        else:
            nc.gpsimd.collective_compute(
                "AllGather", OP.bypass, replica_groups=rg,
                ins=[agx_in[:].opt()], outs=[agx_out[:].opt()])

        # ---------- logits + log_softmax ----------
        with (
            tc.tile_pool(name="fcw", bufs=1) as fcwp,
            tc.tile_pool(name="lx", bufs=2) as lxp,
            tc.tile_pool(name="lt", bufs=2) as ltp,
            tc.tile_pool(name="lps", bufs=4, space="PSUM") as lpsp,
            tc.tile_pool(name="sps", bufs=2, space="PSUM") as spsp,
            tc.tile_pool(name="lac", bufs=4) as lacp,
        ):
            fw = fcwp.tile([128, 8, VS], fr, tag="fw")
            nc.sync.dma_start(fw[:], fcwT[:])
            fb = fcwp.tile([1, VS], fr, tag="fb")
            nc.sync.dma_start(fb[:], fcb[:])

            def finalize(item):
                blk, lt = item
                ag_sb = lacp.tile([8, 128], f32, tag="agsb")
                nc.sync.dma_start(ag_sb[:], ags_out[blk][:])
                sred = spsp.tile([128, 1], f32, tag="sred")
                nc.tensor.matmul(sred[:], ag_sb[:], ones8[:],
                                 start=True, stop=True)
                logS = lacp.tile([128, 1], f32, tag="logS")
                nc.scalar.activation(logS[:], sred[:], AF.Ln)
                nc.gpsimd.tensor_scalar_sub(lt[:], lt[:], logS[:])
                nc.sync.dma_start(out_d[128 * blk:128 * (blk + 1), :], lt[:])

            NCH = VS // 512  # 8 vocab chunks per block
            pend = []
            for blk in range(NBLK if nblk_lim is None else nblk_lim):
                xt = lxp.tile([128, 8, 128], fr, tag="xt")
                nc.sync.dma_start(
                    xt[:],
                    agx_out[128 * (blk // 4):128 * (blk // 4) + 128,
                            :, 128 * (blk % 4):128 * (blk % 4) + 128])
                lt = ltp.tile([128, VS], f32, tag="lt")
                acc = lacp.tile([128, NCH], f32, tag="acc")
                for nich in range(NCH):
                    pb = lpsp.tile([128, 512], f32, tag="pb")
                    for k in range(8):
                        nc.tensor.matmul(
                            pb[:], xt[:, k, :],
                            fw[:, k, 512 * nich:512 * (nich + 1)],
                            start=(k == 0), stop=False)
                    nc.tensor.matmul(
                        pb[:], ones[0:1, 0:128],
                        fb[0:1, 512 * nich:512 * (nich + 1)],
                        start=False, stop=True)
                    # exp(l) straight into the block tile + row-sum
                    nc.scalar.activation(
                        lt[:, 512 * nich:512 * (nich + 1)], pb[:], AF.Exp,
                        accum_out=acc[:, nich:nich + 1])
                sp = lacp.tile([128, 1], f32, tag="sp")
                nc.vector.tensor_reduce(
                    sp[:], acc[:], axis=mybir.AxisListType.X, op=OP.add)
                # restore l = ln(exp(l)); off the AllGather critical path
                nc.scalar.activation(lt[:], lt[:], AF.Ln)
                # partial sums -> [1,128] -> DRAM -> AllGather
                spt = spsp.tile([1, 128], f32, tag="spt")
                nc.tensor.transpose(spt[:], sp[:], ident128[:])
                sps_sb = lacp.tile([1, 128], f32, tag="spsb")
                nc.vector.tensor_copy(sps_sb[:], spt[:])
                nc.sync.dma_start(ags_in[blk][:], sps_sb[:])
                if sim:
                    nc.sync.dma_start(ags_out[blk][0:1], ags_in[blk][:])
                else:
                    nc.gpsimd.collective_compute(
                        "AllGather", OP.bypass, replica_groups=rg,
                        ins=[ags_in[blk][:].opt()],
                        outs=[ags_out[blk][:].opt()])
                pend.append((blk, lt))
                if len(pend) == 2:
                    finalize(pend.pop(0))
            while pend:
                finalize(pend.pop(0))

    nc.compile()
    return nc


def _get_nc():
    if "nc" not in _BUILT:
        _BUILT["nc"] = _build_nc()
    return _BUILT["nc"]


def _prep_inputs(inputs):
    """Host-side shard + relayout. Returns in_maps for 8 cores."""
    tgt = np.asarray(inputs["target"])
    ctx = np.asarray(inputs["context"], np.float32)
    emb_t = np.asarray(inputs["embed_table"], np.float32)
    fc_w = np.asarray(inputs["fc_w"], np.float32)
    fc_b = np.asarray(inputs["fc_b"], np.float32)

    def wT(w, kc):     # [2, G, I] -> [128, kc, 2, G]
        return np.ascontiguousarray(
            w.transpose(2, 0, 1).reshape(kc, 128, 2, G).transpose(1, 0, 2, 3))

    gmask_rz = (np.arange(G) < 2 * H)

    def biases(b_ih, b_hh):
        bgi = b_ih + np.where(gmask_rz[None, :], b_hh, 0.0)
        bhh = np.where(gmask_rz[None, :], 0.0, b_hh)
        return (np.ascontiguousarray(bgi[None], np.float32),
                np.ascontiguousarray(bhh[None], np.float32))

    w_ih0 = np.asarray(inputs["w_ih0"], np.float32)
    w_hh0 = np.asarray(inputs["w_hh0"], np.float32)
    w_ih1 = np.asarray(inputs["w_ih1"], np.float32)
    w_hh1 = np.asarray(inputs["w_hh1"], np.float32)
    WihT0 = wT(w_ih0, 4)
    WhhT0 = wT(w_hh0, 4)
    WihT1 = wT(w_ih1, 8)
    WhhT1 = wT(w_hh1, 4)
    bGi0, bHh0 = biases(np.asarray(inputs["b_ih0"], np.float32),
                        np.asarray(inputs["b_hh0"], np.float32))
    bGi1, bHh1 = biases(np.asarray(inputs["b_ih1"], np.float32),
                        np.asarray(inputs["b_hh1"], np.float32))

    fc_w_pad = np.zeros((VPAD, 2 * H), np.float32)
    fc_w_pad[:V] = fc_w
    fc_b_pad = np.full((VPAD,), NEG, np.float32)
    fc_b_pad[:V] = fc_b

    emb = emb_t[tgt]                      # [B, S, E]
    ctx4 = ctx.reshape(L, 2, B, H)        # [l, d, b, h]

    in_maps = []
    for c in range(NC_):
        bs = slice(BPC * c, BPC * (c + 1))
        emb_rows = emb[bs].transpose(1, 0, 2).reshape(R, E)   # row = 4t+b
        embT = np.ascontiguousarray(
            emb_rows.T.reshape(4, 128, R).transpose(1, 0, 2))
        cc = ctx4[:, :, bs, :]                                # [l, d, 4, h]
        h0T = np.ascontiguousarray(
            cc.transpose(3, 0, 1, 2).reshape(4, 128, L, 2, BPC)
            .transpose(1, 0, 2, 3, 4))
        shard = fc_w_pad[VS * c:VS * (c + 1)]                  # [VS, 1024]
        fcwT = np.ascontiguousarray(
            shard.T.reshape(8, 128, VS).transpose(1, 0, 2))
        fcb = np.ascontiguousarray(fc_b_pad[None, VS * c:VS * (c + 1)])
        in_maps.append({
            "embT": embT, "h0T": h0T,
            "WihT0": WihT0, "WhhT0": WhhT0, "bGi0": bGi0, "bHh0": bHh0,
            "WihT1": WihT1, "WhhT1": WhhT1, "bGi1": bGi1, "bHh1": bHh1,
            "fcwT": fcwT, "fcb": fcb,
            "ones": np.ones((1, 512), np.float32),
            "id4": np.eye(4, dtype=np.float32),
        })
    return in_maps


def _unshard(results):
    Lfull = np.concatenate([results[c]["out"] for c in range(NC_)], axis=1)
    Lfull = Lfull[:, :V]                  # [4096, 32000]
    b = np.arange(B)[:, None]
    s = np.arange(S)[None, :]
    rows = (b // BPC) * R + BPC * s + (b % BPC)
    return Lfull[rows]                    # [B, S, V]


def kernel(**inputs):
    from concourse.bass_utils import run_bass_kernel_spmd
    nc = _get_nc()
    in_maps = _prep_inputs(inputs)
    res = run_bass_kernel_spmd(nc, in_maps, core_ids=list(range(NC_)))
    return _unshard(res.results)



# revision 3
# speedup vs baseline: 6.5051x; 6.5051x over previous
"""Trainium2 Bass kernel for nn_PlainDecoder (2-layer 2-dir GRU decoder +
vocab projection + log_softmax).

Sharding: data-parallel over batch (4 per core) for the scan; vocab-parallel
(4096-wide shard of padded 32768) for the logits.

Scan design (transposed orientation): all gate matmuls output
[128 gate-partitions, 4 batch] so the PE bill (prop. to output FREE size) is
tiny and h' is produced directly in lhsT (hidden-major) layout -- no PE
transposes.  Per (layer, dir) a windowed PSUM tile P holds, per step, 16
slots of 128 gates: [0:8]=r|z (preloaded with gi+bias), [8:12]=n-gh
(preloaded with b_hh_n), [12:16]=gi_n (+b_ih_n).  A bias matmul opens each
window bank (start=True), the windowed gi GEMM and the per-step Whh matmuls
accumulate on top (start=False).  Both layers run interleaved (layer 1 lags
LAG steps).  Everything bf16 into the PE, f32 in PSUM.

Logits: x2 (= layer-1 hist, bf16) is scaled to fp8 and AllGather'd; fc_w is
fp8.  Matmuls run in DoubleRow perf mode (K=256/instr, 0.5 cyc/row).  Per
(128-row block, 1024-vocab chunk): exp(l/256) with accumulated row sums and
a bf16 copy of l (frees PSUM fast); one AllReduce of partial sums per block;
out = l - ln(S) written bf16 (host converts to f32).
"""

import os
import sys
from contextlib import ExitStack

for _p in ("/opt/trn_rl_repo", "/root/.axon_site/_ro/trn_rl_repo"):
    if os.path.isdir(_p) and _p not in sys.path:
        sys.path.insert(0, _p)

import numpy as np  # noqa: E402
import ml_dtypes  # noqa: E402

V, E, H, L, B, S = 32000, 512, 512, 2, 32, 128
NC_ = 8                      # cores
BPC = B // NC_               # batches per core = 4
R = BPC * S                  # rows per core = 512 (s-major: row = 4*t + b)
VPAD = 32768
VS = VPAD // NC_             # vocab shard per core = 4096
W = 8                        # scan PSUM window (steps)
LAG = 12                     # layer-1 lag (steps)
NW = S // W                  # 16 windows
SW = 64.0                    # fc_w fp8 scale
SX = 4.0                     # x2 fp8 scale
SREC = 1.0 / (SW * SX)       # logits descale
PADB = -240.0                # pad-vocab scaled bias (e4m3 max finite)
NROW = NC_ * R               # 4096 global rows
NBLK = NROW // 128           # 32 row blocks

_BUILT = {}


def _build_nc(n_cores=NC_, sim=False):
    """Build the Bass program (same NEFF for all cores; per-core data
    differs).  sim=True replaces collectives with local DMAs so TimelineSim
    can run."""
    import concourse.bass as bass  # noqa: F401
    import concourse.mybir as mybir
    import concourse.tile as tile
    from concourse import bacc

    dt = mybir.dt
    f32 = dt.float32
    bf = dt.bfloat16
    f8 = dt.float8e4
    AF = mybir.ActivationFunctionType
    OP = mybir.AluOpType
    PM = mybir.MatmulPerfMode

    nc = bacc.Bacc("TRN2", target_bir_lowering=False, debug=False,
                   num_devices=n_cores)

    # ---------------- DRAM I/O ----------------
    embT = nc.dram_tensor("embT", [128, 4, R], bf, kind="ExternalInput")
    h0T = nc.dram_tensor("h0T", [128, 2, 2, 4, BPC], bf, kind="ExternalInput")
    WihT0 = nc.dram_tensor("WihT0", [128, 4, 2, 12, 128], bf,
                           kind="ExternalInput")
    WhhT0 = nc.dram_tensor("WhhT0", [128, 4, 2, 12, 128], bf,
                           kind="ExternalInput")
    WihT1 = nc.dram_tensor("WihT1", [128, 8, 2, 12, 128], bf,
                           kind="ExternalInput")
    WhhT1 = nc.dram_tensor("WhhT1", [128, 4, 2, 12, 128], bf,
                           kind="ExternalInput")
    biasT = nc.dram_tensor("biasT", [16, 2, 2, 128], bf, kind="ExternalInput")
    sel16 = nc.dram_tensor("sel16", [16, 16, W, BPC], bf, kind="ExternalInput")
    fcw8 = nc.dram_tensor("fcw8", [128, 8, VS], f8, kind="ExternalInput")
    fcb8 = nc.dram_tensor("fcb8", [1, 4, 2, 1024], f8, kind="ExternalInput")
    ones8 = nc.dram_tensor("ones8", [1, 2, 128], f8, kind="ExternalInput")

    out_d = nc.dram_tensor("out", [NROW, VS], bf, kind="ExternalOutput")

    # internal DRAM for collectives
    agx_in = nc.dram_tensor("agx_in", [128, 8, R], f8, kind="Internal")
    agx_out = nc.dram_tensor("agx_out", [n_cores * 128, 8, R], f8,
                             kind="Internal", addr_space="Shared")
    ags_in = [nc.dram_tensor(f"ags_in{g}", [128, 1], f32, kind="Internal")
              for g in range(NBLK)]
    ags_out = [nc.dram_tensor(f"ags_out{g}", [128, 1], f32,
                              kind="Internal", addr_space="Shared")
               for g in range(NBLK)]
    rg = [list(range(n_cores))]

    with tile.TileContext(nc) as tc, ExitStack() as top:
        # ---------------- scan phase ----------------
        with ExitStack() as scan_stack:
            wpool = scan_stack.enter_context(tc.tile_pool(name="wts", bufs=1))
            hpool = scan_stack.enter_context(tc.tile_pool(name="hist", bufs=1))
            cpool = scan_stack.enter_context(tc.tile_pool(name="chain", bufs=3))
            p0pool = scan_stack.enter_context(
                tc.tile_pool(name="P0", bufs=2, space="PSUM"))
            p1pool = scan_stack.enter_context(
                tc.tile_pool(name="P1", bufs=2, space="PSUM"))

            embT_sb = wpool.tile([128, 4, R], bf, tag="embT", name="embT_sb")
            nc.sync.dma_start(embT_sb[:], embT[:])
            h0_sb = wpool.tile([128, 2, 2, 4, BPC], bf, tag="h0", name="h0_sb")
            nc.sync.dma_start(h0_sb[:], h0T[:])
            wih0 = wpool.tile([128, 4, 2, 12, 128], bf, tag="wih0",
                              name="wih0")
            nc.sync.dma_start(wih0[:], WihT0[:])
            whh0 = wpool.tile([128, 4, 2, 12, 128], bf, tag="whh0",
                              name="whh0")
            nc.sync.dma_start(whh0[:], WhhT0[:])
            wih1 = wpool.tile([128, 8, 2, 12, 128], bf, tag="wih1",
                              name="wih1")
            nc.sync.dma_start(wih1[:], WihT1[:])
            whh1 = wpool.tile([128, 4, 2, 12, 128], bf, tag="whh1",
                              name="whh1")
            nc.sync.dma_start(whh1[:], WhhT1[:])
            bias_sb = wpool.tile([16, 2, 2, 128], bf, tag="bias",
                                 name="bias_sb")
            nc.sync.dma_start(bias_sb[:], biasT[:])
            sel_sb = wpool.tile([16, 16, W, BPC], bf, tag="sel", name="sel_sb")
            nc.sync.dma_start(sel_sb[:], sel16[:])

            # hist layout: [128 h-part, dir, kchunk, row(=4t+b)]
            hist = [hpool.tile([128, 2, 4, R], bf, tag=f"hist{l}",
                               name=f"hist{l}") for l in range(2)]
            ppool = [p0pool, p1pool]
            wih = [wih0, wih1]
            whh = [whh0, whh1]
            kc_ih = [4, 8]
            pwin = [{}, {}]       # (layer, window) -> PSUM tile

            def gi_window(l, w):
                """Bias opener + gi GEMM for window w of layer l.
                P layout: [128, dir, slot16, W, BPC]."""
                P = ppool[l].tile([128, 2, 16, W, BPC], f32, tag=f"P{l}",
                                  name=f"P{l}w{w}")
                pwin[l][w] = P
                rows = slice(BPC * W * w, BPC * W * (w + 1))
                for d in range(2):
                    nc.tensor.matmul(P[:, d], bias_sb[:, l, d, :], sel_sb[:],
                                     start=True, stop=False,
                                     skip_group_check=True)
                for d in range(2):
                    for sl in range(12):
                        slot = sl if sl < 8 else sl + 4
                        for k in range(kc_ih[l]):
                            if l == 0:
                                rhs = embT_sb[:, k, rows]
                            else:
                                rhs = hist[0][:, k // 4, k % 4, rows]
                            nc.tensor.matmul(
                                P[:, d, slot], wih[l][:, k, d, sl, :], rhs,
                                start=False,
                                stop=(slot >= 12 and k == kc_ih[l] - 1),
                                skip_group_check=True)

            def scan_step(l, t):
                """Whh matmuls + GRU cell chain for step t of layer l."""
                P = pwin[l][t // W]
                tw = t % W
                if t == 0:
                    hp = h0_sb[:, l]                      # [128, 2, 4, BPC]
                else:
                    hp = hist[l][:, :, :, BPC * (t - 1):BPC * t]
                for d in range(2):
                    for j in range(12):
                        for k in range(4):
                            nc.tensor.matmul(
                                P[:, d, j, tw, :], whh[l][:, k, d, j, :],
                                hp[:, d, k, :], start=False,
                                stop=(k == 3), skip_group_check=True)
                # GRU cell chain (both dirs in one op each)
                rzs = cpool.tile([128, 2, 8, BPC], f32, tag=f"rzs{l}",
                                 name=f"rzs{l}")
                nc.scalar.activation(rzs[:], P[:, :, 0:8, tw, :], AF.Sigmoid)
                n1 = cpool.tile([128, 2, 4, BPC], f32, tag=f"n1{l}",
                                name=f"n1{l}")
                nc.vector.tensor_mul(n1[:], P[:, :, 8:12, tw, :],
                                     rzs[:, :, 0:4, :])
                nc.vector.tensor_add(n1[:], n1[:], P[:, :, 12:16, tw, :])
                nt = cpool.tile([128, 2, 4, BPC], f32, tag=f"nt{l}",
                                name=f"nt{l}")
                nc.scalar.activation(nt[:], n1[:], AF.Tanh)
                # h' = nt + z*(hprev - nt)  (SBUF operands only)
                d1 = cpool.tile([128, 2, 4, BPC], f32, tag=f"d1{l}",
                                name=f"d1{l}")
                eng = nc.vector if l == 0 else nc.gpsimd
                eng.tensor_sub(d1[:], hp[:], nt[:])
                eng.tensor_mul(d1[:], d1[:], rzs[:, :, 4:8, :])
                eng.tensor_add(hist[l][:, :, :, BPC * t:BPC * (t + 1)],
                               d1[:], nt[:])

            gi_window(0, 0)
            for it in range(S + LAG):
                if it % W == 5 and (it + 3) // W < NW:
                    gi_window(0, (it + 3) // W)
                if it % W == 1 and it >= 9 and (it - 9) // W < NW:
                    gi_window(1, (it - 9) // W)
                if it < S:
                    scan_step(0, it)
                t1 = it - LAG
                if 0 <= t1 < S:
                    scan_step(1, t1)

            # x2 -> fp8, ship out for the AllGather
            x8 = cpool.tile([128, 2, 4, R], f8, tag="x8", bufs=1, name="x8")
            nc.vector.tensor_scalar_mul(x8[:], hist[1][:], SX)
            nc.sync.dma_start(agx_in[:],
                              x8[:].rearrange("p d k r -> p (d k) r"))

        if sim:
            nc.sync.dma_start(agx_out[0:128], agx_in[:])
        else:
            nc.gpsimd.collective_compute(
                "AllGather", OP.bypass, replica_groups=rg,
                ins=[agx_in[:].opt()], outs=[agx_out[:].opt()])

        # ---------------- logits + log_softmax ----------------
        with (
            tc.tile_pool(name="fw", bufs=1) as fwpool,
            tc.tile_pool(name="lt", bufs=3) as ltpool,
            tc.tile_pool(name="lps", bufs=4, space="PSUM") as lpspool,
            tc.tile_pool(name="lsc", bufs=2) as lscpool,
        ):
            fw = fwpool.tile([128, 8, VS], f8, tag="fw", name="fw")
            nc.sync.dma_start(fw[:], fcw8[:])
            fb = fwpool.tile([1, 4, 2, 1024], f8, tag="fb", name="fb")
            nc.sync.dma_start(fb[:], fcb8[:])
            on8 = fwpool.tile([1, 2, 128], f8, tag="on8", name="on8")
            nc.sync.dma_start(on8[:], ones8[:])
            x2g = fwpool.tile([128, 8, 8, R], f8, tag="x2g", name="x2g")
            nc.sync.dma_start(
                x2g[:], agx_out[:].rearrange("(c p) k r -> p k c r", p=128))

            prev_out = [None]

            def block(rb):
                csrc, r0 = rb // BPC, (rb % BPC) * 128
                lb = ltpool.tile([128, 4, 1024], bf, tag="lb", name="lb")
                ob = ltpool.tile([128, VS], bf, tag="ob", name="ob")
                srb = lscpool.tile([128, 4], f32, tag="srb", name="srb")
                for vq in range(4):
                    P = lpspool.tile([128, 1024], f32, tag="lp", name="lp")
                    nc.tensor.matmul(P[:], on8[:], fb[0:1, vq], start=True,
                                     stop=False, perf_mode=PM.DoubleRow,
                                     skip_group_check=True)
                    for c2 in range(4):
                        nc.tensor.matmul(
                            P[:], x2g[:, 2 * c2:2 * c2 + 2, csrc,
                                      r0:r0 + 128],
                            fw[:, 2 * c2:2 * c2 + 2,
                               1024 * vq:1024 * (vq + 1)],
                            start=False, stop=(c2 == 3),
                            perf_mode=PM.DoubleRow, skip_group_check=True)
                    eb = lscpool.tile([128, 1024], bf, tag="eb", name="eb")
                    nc.scalar.activation(eb[:], P[:], AF.Exp, scale=SREC,
                                         accum_out=srb[:, vq:vq + 1])
                    # bf16 copy of l (frees PSUM): vq0 on ACT, rest on DVE
                    if vq == 0:
                        nc.scalar.mul(lb[:, vq, :], P[:], SREC)
                    else:
                        nc.vector.tensor_scalar_mul(lb[:, vq, :], P[:], SREC)
                s1 = lscpool.tile([128, 1], f32, tag="s1", name="s1")
                nc.vector.tensor_reduce(s1[:], srb[:],
                                        axis=mybir.AxisListType.X, op=OP.add)
                # write previous block's output before touching SP with the
                # collective chain for this block
                if prev_out[0] is not None:
                    prb, pob = prev_out[0]
                    nc.sync.dma_start(out_d[128 * prb:128 * (prb + 1), :],
                                      pob[:])
                nc.sync.dma_start(ags_in[rb][:], s1[:])
                if sim:
                    nc.sync.dma_start(ags_out[rb][:], ags_in[rb][:])
                else:
                    nc.gpsimd.collective_compute(
                        "AllReduce", OP.add, replica_groups=rg,
                        ins=[ags_in[rb][:].opt()],
                        outs=[ags_out[rb][:].opt()])
                sg = lscpool.tile([128, 1], f32, tag="sg", name="sg")
                nc.sync.dma_start(sg[:], ags_out[rb][:])
                lnS = lscpool.tile([128, 1], f32, tag="lnS", name="lnS")
                nc.scalar.activation(lnS[:], sg[:], AF.Ln)
                for vq in range(4):
                    eng = nc.vector if vq < 2 else nc.gpsimd
                    eng.tensor_scalar_sub(ob[:, 1024 * vq:1024 * (vq + 1)],
                                          lb[:, vq, :], lnS[:])
                prev_out[0] = (rb, ob)

            for rb in range(NBLK):
                block(rb)
            prb, pob = prev_out[0]
            nc.sync.dma_start(out_d[128 * prb:128 * (prb + 1), :], pob[:])

    nc.compile()
    return nc


def _get_nc():
    if "nc" not in _BUILT:
        _BUILT["nc"] = _build_nc()
    return _BUILT["nc"]


def _prep_inputs(inputs):
    """Host-side shard + relayout. Returns in_maps for 8 cores."""
    bft = ml_dtypes.bfloat16
    f8t = ml_dtypes.float8_e4m3

    tgt = np.asarray(inputs["target"])
    ctx = np.asarray(inputs["context"], np.float32)
    emb_t = np.asarray(inputs["embed_table"], np.float32)
    fc_w = np.asarray(inputs["fc_w"], np.float32)
    fc_b = np.asarray(inputs["fc_b"], np.float32)

    def wT(w, kc):     # [2, 1536, IN] -> [128, kc, 2, 12, 128]
        w = np.asarray(w, np.float32)
        a = w.transpose(2, 0, 1).reshape(kc, 128, 2, 12, 128)
        return np.ascontiguousarray(a.transpose(1, 0, 2, 3, 4)).astype(bft)

    WihT0 = wT(inputs["w_ih0"], 4)
    WhhT0 = wT(inputs["w_hh0"], 4)
    WihT1 = wT(inputs["w_ih1"], 8)
    WhhT1 = wT(inputs["w_hh1"], 4)

    # biasT[slot, layer, dir, g]
    biasT = np.zeros((16, 2, 2, 128), np.float32)
    for l, (bi, bh) in enumerate([
            (np.asarray(inputs["b_ih0"], np.float32),
             np.asarray(inputs["b_hh0"], np.float32)),
            (np.asarray(inputs["b_ih1"], np.float32),
             np.asarray(inputs["b_hh1"], np.float32))]):
        for d in range(2):
            rz = (bi[d, :1024] + bh[d, :1024]).reshape(8, 128)
            biasT[0:8, l, d, :] = rz
            biasT[8:12, l, d, :] = bh[d, 1024:].reshape(4, 128)
            biasT[12:16, l, d, :] = bi[d, 1024:].reshape(4, 128)
    biasT = biasT.astype(bft)

    sel = np.zeros((16, 16, W, BPC), np.float32)
    for s in range(16):
        sel[s, s] = 1.0
    sel = sel.astype(bft)

    fcw_pad = np.zeros((VPAD, 2 * H), np.float32)
    fcw_pad[:V] = fc_w
    fcb_pad = np.full((VPAD,), PADB, np.float32)
    fcb_pad[:V] = fc_b * (SW * SX)

    ones8 = np.zeros((1, 2, 128), np.float32)
    ones8[0, 0, :] = 1.0
    ones8 = ones8.astype(f8t)

    emb = emb_t[tgt]                      # [B, S, E]
    ctx4 = ctx.reshape(L, 2, B, H)        # [l, d, b, h]

    in_maps = []
    for c in range(NC_):
        bs = slice(BPC * c, BPC * (c + 1))
        emb_rows = emb[bs].transpose(1, 0, 2).reshape(R, E)   # row = 4t+b
        embT = np.ascontiguousarray(
            emb_rows.T.reshape(4, 128, R).transpose(1, 0, 2)).astype(bft)
        cc = ctx4[:, :, bs, :]                                # [l, d, 4, h]
        h0a = cc.transpose(3, 0, 1, 2).reshape(4, 128, L, 2, BPC)
        h0T = np.ascontiguousarray(
            h0a.transpose(1, 2, 3, 0, 4)).astype(bft)
        shard = fcw_pad[VS * c:VS * (c + 1)] * SW             # [VS, 1024]
        fcw8 = np.ascontiguousarray(
            shard.T.reshape(8, 128, VS).transpose(1, 0, 2)).astype(f8t)
        fcb8 = np.zeros((1, 4, 2, 1024), np.float32)
        fcb8[0, :, 0, :] = fcb_pad[VS * c:VS * (c + 1)].reshape(4, 1024)
        fcb8 = fcb8.astype(f8t)
        in_maps.append({
            "embT": embT, "h0T": h0T,
            "WihT0": WihT0, "WhhT0": WhhT0,
            "WihT1": WihT1, "WhhT1": WhhT1,
            "biasT": biasT, "sel16": sel,
            "fcw8": fcw8, "fcb8": fcb8, "ones8": ones8,
        })
    return in_maps


def _unshard(results):
    Lfull = np.concatenate(
        [results[c]["out"].astype(np.float32) for c in range(NC_)], axis=1)
    Lfull = Lfull[:, :V]                  # [4096, 32000]
    b = np.arange(B)[:, None]
    s = np.arange(S)[None, :]
    rows = (b // BPC) * R + BPC * s + (b % BPC)
    return Lfull[rows]                    # [B, S, V]


def kernel(**inputs):
    from concourse.bass_utils import run_bass_kernel_spmd
    nc = _get_nc()
    in_maps = _prep_inputs(inputs)
    res = run_bass_kernel_spmd(nc, in_maps, core_ids=list(range(NC_)))
    return _unshard(res.results)
